# revision 12
# baseline (speedup 1.0000x reference)
"""Differentiable-FK forward kernel for Trainium2 (8 NeuronCores, data-parallel).

Problem: batch B=131072 of kinematic chains (63 bodies: world, free root,
61 hinges), 16 sites gathered from bodies. Output [B, 16, 3] site positions.

Strategy: pure data-parallel across 8 cores (16384 rows each). Per core the
batch is laid out as [128 partitions x 128 free]. The hinge chain is evaluated
sequentially (61 quaternion composes). Positions use a telescoped accumulation

    out_s(m) = wp1 + Rt(wq_1)K_2 + sum_{j=2}^{m-1} Rt(wq_j) G_j
             + Rt(wq_m)(sp_s - jp_m) + CONST_s

where Rt(q)v = R(q)v - v is the pure-quadratic part of the rotation and all
constant vectors (G_j, CONST_s) are host-precomputed from the tiny tree
tensors and baked into instruction immediates. The kernel is recompiled (and
NEFF-cached) per unique set of tree constants; qpos is the only streamed
input.

Quaternion state lives in [128, 4*F] block tiles (component-major columns) so
the 16 per-body Hamilton products collapse to 4 broadcast tensor_tensor ops
and the 12 combine adds to 6 column-pair ops; rotation emits use a cyclically
ordered cross-product tile so each emit is ~15 instructions of [128, 2-3*F]
blocks. Work is spread across DVE / Pool / Act with a size-aware greedy
balancer using HW-calibrated per-op costs.
"""
import hashlib
import os
import numpy as np

# engine-mix tuning knob: scales Pool's modeled cost (higher -> less on Pool;
# 1e9 disables Pool entirely). HW A/B'd via FK_POOL_FACTOR.
_POOL_FACTOR = float(os.environ.get("FK_POOL_FACTOR", "1.0"))

import concourse.bacc as bacc
import concourse.mybir as mybir
from concourse.tile import TileContext
from concourse.bass_utils import run_bass_kernel_spmd

F32 = mybir.dt.float32
MULT = mybir.AluOpType.mult
ADD = mybir.AluOpType.add
SUB = mybir.AluOpType.subtract

B_FULL = 131072
NCORES = 8
N = B_FULL // NCORES          # 16384 rows per core
P = 128                       # partitions
F = N // P                    # 128 free
NBODY = 63
NH = NBODY - 2                # 61
NQ = 7 + NH                   # 68
NSITES = 16
OUTW = NSITES * 3             # 48

HALF_PI = float(np.pi / 2)

# HW-calibrated per-op costs (ns), indexed by free-size multiple k (k*128).
DVE_TT = {1: 229.0, 2: 366.0, 3: 504.0, 4: 641.0}
POOL_TT = {1: 436.0, 2: 718.0, 3: 1000.0, 4: 1283.0}
DVE_TS = 197.0
ACT_TS = 326.0
DVE_STT = 288.0
DVE_CP = {1: 197.0, 2: 330.0, 3: 460.0, 4: 590.0}
ACT_CP = {1: 326.0, 2: 450.0, 3: 580.0, 4: 710.0}

_CACHE = {}


def _qmul_np(q1, q2):
    w1, x1, y1, z1 = [q1[..., i] for i in range(4)]
    w2, x2, y2, z2 = [q2[..., i] for i in range(4)]
    return np.stack([
        w1 * w2 - x1 * x2 - y1 * y2 - z1 * z2,
        w1 * x2 + x1 * w2 + y1 * z2 - z1 * y2,
        w1 * y2 - x1 * z2 + y1 * w2 + z1 * x2,
        w1 * z2 + x1 * y2 - y1 * x2 + z1 * w2,
    ], -1)


class Bal:
    """Greedy size-aware engine balancer across DVE / Pool / Act."""

    def __init__(self, nc):
        self.nc = nc
        self.load = {"dve": 0.0, "pool": 0.0, "act": 0.0}

    def tt(self, out, a, b, op, k=1):
        dve_c = DVE_TT[k]
        pool_c = POOL_TT[k] * _POOL_FACTOR
        if self.load["dve"] + dve_c <= self.load["pool"] + pool_c:
            self.load["dve"] += dve_c
            self.nc.vector.tensor_tensor(out, a, b, op)
        else:
            self.load["pool"] += pool_c
            self.nc.gpsimd.tensor_tensor(out, a, b, op)

    def stt(self, out, a, scal, b, op0, op1):
        self.load["dve"] += DVE_STT
        self.nc.vector.scalar_tensor_tensor(out, a, scal, b, op0, op1)

    def ts_mult(self, out, a, scal):
        if self.load["dve"] + DVE_TS <= self.load["act"] + ACT_TS:
            self.load["dve"] += DVE_TS
            self.nc.vector.tensor_scalar(out, a, float(scal), None, MULT)
        else:
            self.load["act"] += ACT_TS
            self.nc.scalar.activation(out, a, mybir.ActivationFunctionType.Copy,
                                      bias=0.0, scale=float(scal))

    def cp(self, out, a, k=1):
        if self.load["dve"] + DVE_CP[k] <= self.load["act"] + ACT_CP[k]:
            self.load["dve"] += DVE_CP[k]
            self.nc.vector.tensor_copy(out, a)
        else:
            self.load["act"] += ACT_CP[k]
            self.nc.scalar.activation(out, a, mybir.ActivationFunctionType.Copy,
                                      bias=0.0, scale=1.0)


def _build(body_pos, body_quat, hinge_axis, jnt_pos, site_pos,
           body_parent, site_body, loop_iters=None):
    # ---- host constant precompute ----
    parents = np.asarray(body_parent).astype(np.int64)
    sbody = np.asarray(site_body).astype(np.int64)
    assert np.array_equal(parents, np.maximum(np.arange(NBODY) - 1, 0)), \
        "kernel specialized for chain topology"

    A = np.asarray(body_quat[2:], np.float64)                      # [NH,4]
    Bq = _qmul_np(np.asarray(body_quat[2:], np.float64),
                  np.concatenate([np.zeros((NH, 1)), np.asarray(hinge_axis, np.float64)], -1))
    K = np.asarray(body_pos[2:], np.float64) + np.asarray(jnt_pos, np.float64)
    jp = np.asarray(jnt_pos, np.float64)
    sp = np.asarray(site_pos, np.float64)

    G = np.zeros((NH, 3))
    for h in range(NH):
        G[h] = (K[h + 1] if h + 1 < NH else 0.0) - jp[h]
    Cpre = np.zeros((NBODY, 3))
    acc = K[0].copy()
    for m in range(2, NBODY):
        Cpre[m] = acc
        acc = acc + G[m - 2]

    site_by_body = {}
    for s, m in enumerate(sbody):
        site_by_body.setdefault(int(m), []).append(s)
    m_max = max(site_by_body.keys())

    # ---- bass program ----
    nc = bacc.Bacc("TRN2")
    qpos_d = nc.dram_tensor("qpos", [N, NQ], F32, kind="ExternalInput")
    out_d = nc.dram_tensor("sites", [N, OUTW], F32, kind="ExternalOutput")

    Sin, Sqrt = (mybir.ActivationFunctionType.Sin,
                 mybir.ActivationFunctionType.Sqrt)

    with TileContext(nc) as tc:
        with tc.tile_pool(name="main", bufs=1) as pool, \
             tc.tile_pool(name="scratch", bufs=2) as sp_pool:

            bal = Bal(nc)

            # persistent allocations, shared across benchmark-loop iterations
            QP = pool.tile([P, F * NQ], F32)
            QPr = QP[:].rearrange("p (f k) -> p k f", k=NQ)       # [P, 68, F]
            s_all = pool.tile([P, NH * F], F32)
            c_all = pool.tile([P, NH * F], F32)
            s3 = s_all[:].rearrange("p (h f) -> p h f", h=NH)
            c3 = c_all[:].rearrange("p (h f) -> p h f", h=NH)
            halfpi = pool.tile([P, 1], F32, name="halfpi")
            nc.vector.memset(halfpi[:], HALF_PI)
            PaccB = pool.tile([P, 3 * F], F32, name="Pacc")
            PaccV = PaccB[:].rearrange("p (a f) -> p a f", a=3)
            OS = pool.tile([P, F * OUTW], F32)
            OSr = OS[:].rearrange("p (f k) -> p k f", k=OUTW)     # [P, 48, F]

            def big(tag, kf):
                t = sp_pool.tile([P, kf * F], F32, tag=tag, name=tag)
                return t[:].rearrange("p (a f) -> p a f", a=kf)

            def tile(tag):
                return sp_pool.tile([P, F], F32, tag=tag, name=tag)

            for _it in range(loop_iters or 1):
                nc.sync.dma_start(QP[:], qpos_d[:].rearrange("(p f) k -> p (f k)", p=P))
                ang = QPr[:, 7:NQ, :]                              # [P, 61, F]
                # s = sin(theta/2), c = sin(theta/2 + pi/2) = cos(theta/2)
                nc.scalar.activation(s3, ang, Sin, bias=0.0, scale=0.5)
                nc.scalar.activation(c3, ang, Sin, bias=halfpi[:], scale=0.5)
                bal.load["act"] += 2 * 6700.0  # bulk trig occupancy

                # ---- root: wq = normalize(qpos[:, 3:7]) ----
                rq = QPr[:, 3:7, :]                                # [P, 4, F] strided
                SQ = big("sq", 4)
                bal.tt(SQ, rq, rq, MULT, 4)
                s01 = big("s01", 2)
                bal.tt(s01, SQ[:, 0:2, :], SQ[:, 2:4, :], ADD, 2)
                n2 = tile("n2")
                bal.tt(n2, s01[:, 0, :], s01[:, 1, :], ADD)
                rn = sp_pool.tile([P, F], F32, tag="rn", name="rn")
                nc.scalar.activation(rn, n2, Sqrt, bias=0.0, scale=1.0)
                nc.vector.reciprocal(rn, rn)
                WQv = big("wq", 4)
                rnb = rn[:].rearrange("p (a f) -> p a f", a=1).to_broadcast([P, 4, F])
                bal.tt(WQv, rq, rnb, MULT, 4)

                bal.cp(PaccV, QPr[:, 0:3, :], 3)

                def emit_rot(WQv, vecs):
                    """vecs = list of (v3, const3_or_None, out_idx).
                    out_idx None -> Pacc += Rt(q)v ; else OSr cols = Pacc + Rt(q)v + cst.
                    WQv components: w=0, x=1, y=2, z=3. T slots: 0=z, 1=x, 2=y.
                    """
                    for (v, cst, oidx) in vecs:
                        v2 = [2.0 * float(v[i]) for i in range(3)]
                        Tv = big("T", 3)
                        for (slot, ia, ib, ca, cb) in ((0, 1, 2, v2[1], v2[0]),
                                                       (1, 2, 3, v2[2], v2[1]),
                                                       (2, 3, 1, v2[0], v2[2])):
                            m = tile("m")
                            bal.ts_mult(m, WQv[:, ia, :], ca)
                            bal.stt(Tv[:, slot, :], WQv[:, ib, :], -cb, m, MULT, ADD)
                        A1v = big("A1", 3)
                        wb = WQv[:, 0:1, :].to_broadcast([P, 2, F])
                        bal.tt(A1v[:, 0:2, :], Tv[:, 1:3, :], wb, MULT, 2)
                        bal.tt(A1v[:, 2, :], Tv[:, 0, :], WQv[:, 0, :], MULT)
                        A2v = big("A2", 3)
                        bal.tt(A2v[:, 0:2, :], WQv[:, 2:4, :], Tv[:, 0:2, :], MULT, 2)
                        bal.tt(A2v[:, 2, :], WQv[:, 1, :], Tv[:, 2, :], MULT)
                        A4v = big("A4", 3)
                        bal.tt(A4v[:, 0, :], WQv[:, 3, :], Tv[:, 2, :], MULT)
                        bal.tt(A4v[:, 1:3, :], WQv[:, 1:3, :], Tv[:, 0:2, :], MULT, 2)
                        Rv = big("R", 3)
                        bal.tt(Rv, A1v, A2v, ADD, 3)
                        bal.tt(Rv, Rv, A4v, SUB, 3)
                        if oidx is None:
                            bal.tt(PaccV, PaccV, Rv, ADD, 3)
                        else:
                            for ci in range(3):
                                bal.stt(OSr[:, oidx + ci, :], Rv[:, ci, :],
                                        float(cst[ci]), PaccV[:, ci, :], ADD, ADD)

                # sites on body 1 (root)
                for sid in site_by_body.get(1, []):
                    emit_rot(WQv, [(sp[sid], sp[sid], 3 * sid)])

                # P init: += Rt(wq1) K_2
                emit_rot(WQv, [(K[0], None, None)])

                # ---- chain ----
                for j in range(2, m_max + 1):
                    h = j - 2
                    # lq = c*A + s*B into LQ block
                    LQv = big("lq", 4)
                    for i in range(4):
                        ti = tile(f"lqt{i}")
                        bal.ts_mult(ti, s3[:, h, :], float(Bq[h, i]))
                        bal.stt(LQv[:, i, :], c3[:, h, :], float(A[h, i]), ti, MULT, ADD)
                    # products: PA_a[b] = wq_a * lq_b  (4 broadcast block ops)
                    PAs = []
                    for a in range(4):
                        PAv = big(f"pa{a}", 4)
                        wqa = WQv[:, a:a + 1, :].to_broadcast([P, 4, F])
                        bal.tt(PAv, LQv, wqa, MULT, 4)
                        PAs.append(PAv)
                    # combines: nq = wq x lq via sign-grouped column pairs
                    NQv = big("wq", 4)
                    bal.tt(NQv[:, 1::2, :], PAs[0][:, 1::2, :], PAs[1][:, 0::2, :], ADD, 2)
                    bal.tt(NQv[:, 0::2, :], PAs[0][:, 0::2, :], PAs[1][:, 1::2, :], SUB, 2)
                    bal.tt(NQv[:, 1:3, :], NQv[:, 1:3, :], PAs[2][:, 3::-3, :], ADD, 2)
                    bal.tt(NQv[:, 0::3, :], NQv[:, 0::3, :], PAs[2][:, 2:0:-1, :], SUB, 2)
                    bal.tt(NQv[:, 2:4, :], NQv[:, 2:4, :], PAs[3][:, 1::-1, :], ADD, 2)
                    bal.tt(NQv[:, 0:2, :], NQv[:, 0:2, :], PAs[3][:, 3:1:-1, :], SUB, 2)
                    WQv = NQv
                    # sites on body j, then G update
                    vecs = []
                    for sid in site_by_body.get(j, []):
                        v = sp[sid] - jp[h]
                        vecs.append((v, Cpre[j] + v, 3 * sid))
                    if j < m_max:
                        vecs.append((G[h], None, None))
                    emit_rot(WQv, vecs)

                nc.sync.dma_start(out_d[:].rearrange("(p f) k -> p (f k)", p=P), OS[:])

    nc.compile()
    return nc


def _get_nc(inputs, loop_iters=None):
    key_src = b"".join(np.ascontiguousarray(np.asarray(inputs[k])).tobytes()
                       for k in ("body_pos", "body_quat", "hinge_axis", "jnt_pos",
                                 "site_pos", "body_parent", "site_body"))
    key = (hashlib.sha256(key_src).hexdigest(), loop_iters)
    if key not in _CACHE:
        _CACHE[key] = _build(inputs["body_pos"], inputs["body_quat"],
                             inputs["hinge_axis"], inputs["jnt_pos"],
                             inputs["site_pos"], inputs["body_parent"],
                             inputs["site_body"], loop_iters=loop_iters)
    return _CACHE[key]


def kernel(**inputs) -> np.ndarray:
    qpos = np.ascontiguousarray(np.asarray(inputs["qpos"], dtype=np.float32))
    assert qpos.shape == (B_FULL, NQ)
    nc = _get_nc(inputs)
    in_maps = [{"qpos": qpos[c * N:(c + 1) * N]} for c in range(NCORES)]
    res = run_bass_kernel_spmd(nc, in_maps, list(range(NCORES)))
    out = np.concatenate([res.results[c]["sites"] for c in range(NCORES)], axis=0)
    return out.reshape(B_FULL, NSITES, 3)


if __name__ == "__main__":
    import importlib.util
    spec = importlib.util.spec_from_file_location("reference", "/root/problem/reference.py")
    ref = importlib.util.module_from_spec(spec)
    spec.loader.exec_module(ref)
    inputs = {k: np.asarray(v) for k, v in ref.setup_inputs().items()}
    out = kernel(**inputs)
    print("out", out.shape, out.dtype)


# revision 16
# speedup vs baseline: 1.3694x; 1.3694x over previous
"""Differentiable-FK forward kernel for Trainium2 (8 NeuronCores, data-parallel).

Problem: batch B=131072 of kinematic chains (63 bodies: world, free root,
61 hinges), 16 sites gathered from bodies. Output [B, 16, 3] site positions.

Strategy: pure data-parallel across 8 cores (16384 rows each). Per core the
batch is laid out as [128 partitions x 128 free]. The hinge chain is evaluated
sequentially (61 quaternion composes). Positions use a telescoped accumulation

    out_s(m) = wp1 + Rt(wq_1)K_2 + sum_{j=2}^{m-1} Rt(wq_j) G_j
             + Rt(wq_m)(sp_s - jp_m) + CONST_s

where Rt(q)v = R(q)v - v is the pure-quadratic part of the rotation and all
constant vectors (G_j, CONST_s) are host-precomputed from the tiny tree
tensors and baked into instruction immediates. The kernel is recompiled (and
NEFF-cached) per unique set of tree constants; qpos is the only streamed
input.

Quaternion state lives in [128, 4*F] block tiles (component-major columns) so
the 16 per-body Hamilton products collapse to 4 broadcast tensor_tensor ops
and the 12 combine adds to 6 column-pair ops; rotation emits use a cyclically
ordered cross-product tile so each emit is ~15 instructions of [128, 2-3*F]
blocks. Work is spread across DVE / Pool / Act with a size-aware greedy
balancer using HW-calibrated per-op costs.
"""
import hashlib
import os
import numpy as np

# engine-mix tuning knob: scales Pool's modeled cost (higher -> less on Pool;
# 1e9 disables Pool entirely). HW A/B'd via FK_POOL_FACTOR.
_POOL_FACTOR = float(os.environ.get("FK_POOL_FACTOR", "1.0"))

import concourse.bacc as bacc
import concourse.mybir as mybir
from concourse.tile import TileContext
from concourse.bass_utils import run_bass_kernel_spmd

F32 = mybir.dt.float32
F16 = mybir.dt.float16
# Core chain dtype: f16 halves DVE big-op time (390 vs 641 ns @[128,512]);
# boundaries (input, root normalize, output) stay f32. FK_F16=0 reverts.
DT = F16 if os.environ.get("FK_F16", "1") == "1" else F32
MULT = mybir.AluOpType.mult
ADD = mybir.AluOpType.add
SUB = mybir.AluOpType.subtract

B_FULL = 131072
NCORES = 8
N = B_FULL // NCORES          # 16384 rows per core
P = 128                       # partitions
F = N // P                    # 128 free
NBODY = 63
NH = NBODY - 2                # 61
NQ = 7 + NH                   # 68
NSITES = 16
OUTW = NSITES * 3             # 48

HALF_PI = float(np.pi / 2)

# HW-calibrated per-op costs (ns), indexed by free-size multiple k (k*128).
if DT is F16:
    DVE_TT = {1: 255.0, 2: 300.0, 3: 345.0, 4: 390.0}
    DVE_STT = 243.0
else:
    DVE_TT = {1: 229.0, 2: 366.0, 3: 504.0, 4: 641.0}
    DVE_STT = 288.0
POOL_TT = {1: 436.0, 2: 718.0, 3: 1000.0, 4: 1283.0}
DVE_TS = 197.0
ACT_TS = 326.0
DVE_CP = {1: 197.0, 2: 330.0, 3: 460.0, 4: 590.0}
ACT_CP = {1: 326.0, 2: 450.0, 3: 580.0, 4: 710.0}

_CACHE = {}


def _qmul_np(q1, q2):
    w1, x1, y1, z1 = [q1[..., i] for i in range(4)]
    w2, x2, y2, z2 = [q2[..., i] for i in range(4)]
    return np.stack([
        w1 * w2 - x1 * x2 - y1 * y2 - z1 * z2,
        w1 * x2 + x1 * w2 + y1 * z2 - z1 * y2,
        w1 * y2 - x1 * z2 + y1 * w2 + z1 * x2,
        w1 * z2 + x1 * y2 - y1 * x2 + z1 * w2,
    ], -1)


class Bal:
    """Greedy size-aware engine balancer across DVE / Pool / Act."""

    def __init__(self, nc):
        self.nc = nc
        self.load = {"dve": 0.0, "pool": 0.0, "act": 0.0}

    def tt(self, out, a, b, op, k=1):
        dve_c = DVE_TT[k]
        pool_c = POOL_TT[k] * _POOL_FACTOR
        if self.load["dve"] + dve_c <= self.load["pool"] + pool_c:
            self.load["dve"] += dve_c
            self.nc.vector.tensor_tensor(out, a, b, op)
        else:
            self.load["pool"] += pool_c
            self.nc.gpsimd.tensor_tensor(out, a, b, op)

    def stt(self, out, a, scal, b, op0, op1):
        self.load["dve"] += DVE_STT
        self.nc.vector.scalar_tensor_tensor(out, a, scal, b, op0, op1)

    def ts_mult(self, out, a, scal):
        if self.load["dve"] + DVE_TS <= self.load["act"] + ACT_TS:
            self.load["dve"] += DVE_TS
            self.nc.vector.tensor_scalar(out, a, float(scal), None, MULT)
        else:
            self.load["act"] += ACT_TS
            self.nc.scalar.activation(out, a, mybir.ActivationFunctionType.Copy,
                                      bias=0.0, scale=float(scal))

    def cp(self, out, a, k=1):
        if self.load["dve"] + DVE_CP[k] <= self.load["act"] + ACT_CP[k]:
            self.load["dve"] += DVE_CP[k]
            self.nc.vector.tensor_copy(out, a)
        else:
            self.load["act"] += ACT_CP[k]
            self.nc.scalar.activation(out, a, mybir.ActivationFunctionType.Copy,
                                      bias=0.0, scale=1.0)


def _build(body_pos, body_quat, hinge_axis, jnt_pos, site_pos,
           body_parent, site_body, loop_iters=None):
    # ---- host constant precompute ----
    parents = np.asarray(body_parent).astype(np.int64)
    sbody = np.asarray(site_body).astype(np.int64)
    assert np.array_equal(parents, np.maximum(np.arange(NBODY) - 1, 0)), \
        "kernel specialized for chain topology"

    A = np.asarray(body_quat[2:], np.float64)                      # [NH,4]
    Bq = _qmul_np(np.asarray(body_quat[2:], np.float64),
                  np.concatenate([np.zeros((NH, 1)), np.asarray(hinge_axis, np.float64)], -1))
    K = np.asarray(body_pos[2:], np.float64) + np.asarray(jnt_pos, np.float64)
    jp = np.asarray(jnt_pos, np.float64)
    sp = np.asarray(site_pos, np.float64)

    G = np.zeros((NH, 3))
    for h in range(NH):
        G[h] = (K[h + 1] if h + 1 < NH else 0.0) - jp[h]
    Cpre = np.zeros((NBODY, 3))
    acc = K[0].copy()
    for m in range(2, NBODY):
        Cpre[m] = acc
        acc = acc + G[m - 2]

    site_by_body = {}
    for s, m in enumerate(sbody):
        site_by_body.setdefault(int(m), []).append(s)
    m_max = max(site_by_body.keys())

    # ---- bass program ----
    nc = bacc.Bacc("TRN2")
    qpos_d = nc.dram_tensor("qpos", [N, NQ], F32, kind="ExternalInput")
    out_d = nc.dram_tensor("sites", [N, OUTW], F32, kind="ExternalOutput")

    Sin, Sqrt = (mybir.ActivationFunctionType.Sin,
                 mybir.ActivationFunctionType.Sqrt)

    with TileContext(nc) as tc:
        with tc.tile_pool(name="main", bufs=1) as pool, \
             tc.tile_pool(name="scratch", bufs=2) as sp_pool:

            bal = Bal(nc)

            # persistent allocations, shared across benchmark-loop iterations
            QP = pool.tile([P, F * NQ], F32)
            QPr = QP[:].rearrange("p (f k) -> p k f", k=NQ)       # [P, 68, F]
            s_all = pool.tile([P, NH * F], DT)
            c_all = pool.tile([P, NH * F], DT)
            s3 = s_all[:].rearrange("p (h f) -> p h f", h=NH)
            c3 = c_all[:].rearrange("p (h f) -> p h f", h=NH)
            halfpi = pool.tile([P, 1], F32, name="halfpi")
            nc.vector.memset(halfpi[:], HALF_PI)
            PaccB = pool.tile([P, 3 * F], DT, name="Pacc")
            PaccV = PaccB[:].rearrange("p (a f) -> p a f", a=3)
            OS = pool.tile([P, F * OUTW], F32)
            OSr = OS[:].rearrange("p (f k) -> p k f", k=OUTW)     # [P, 48, F]

            def big(tag, kf, dt=DT):
                t = sp_pool.tile([P, kf * F], dt, tag=tag, name=tag)
                return t[:].rearrange("p (a f) -> p a f", a=kf)

            def tile(tag, dt=DT):
                return sp_pool.tile([P, F], dt, tag=tag, name=tag)

            for _it in range(loop_iters or 1):
                nc.sync.dma_start(QP[:], qpos_d[:].rearrange("(p f) k -> p (f k)", p=P))
                ang = QPr[:, 7:NQ, :]                              # [P, 61, F]
                # s = sin(theta/2), c = sin(theta/2 + pi/2) = cos(theta/2)
                nc.scalar.activation(s3, ang, Sin, bias=0.0, scale=0.5)
                nc.scalar.activation(c3, ang, Sin, bias=halfpi[:], scale=0.5)
                bal.load["act"] += 2 * 6700.0  # bulk trig occupancy

                # ---- root: wq = normalize(qpos[:, 3:7]) ----
                rq = QPr[:, 3:7, :]                                # [P, 4, F] strided
                SQ = big("sq", 4, F32)
                bal.tt(SQ, rq, rq, MULT, 4)
                s01 = big("s01", 2, F32)
                bal.tt(s01, SQ[:, 0:2, :], SQ[:, 2:4, :], ADD, 2)
                n2 = tile("n2", F32)
                bal.tt(n2, s01[:, 0, :], s01[:, 1, :], ADD)
                rn = sp_pool.tile([P, F], F32, tag="rn", name="rn")
                nc.scalar.activation(rn, n2, Sqrt, bias=0.0, scale=1.0)
                nc.vector.reciprocal(rn, rn)
                WQv = big("wq", 4)
                rnb = rn[:].rearrange("p (a f) -> p a f", a=1).to_broadcast([P, 4, F])
                bal.tt(WQv, rq, rnb, MULT, 4)

                bal.cp(PaccV, QPr[:, 0:3, :], 3)

                def emit_rot(WQv, vecs):
                    """vecs = list of (v3, const3_or_None, out_idx).
                    out_idx None -> Pacc += Rt(q)v ; else OSr cols = Pacc + Rt(q)v + cst.
                    WQv components: w=0, x=1, y=2, z=3. T slots: 0=z, 1=x, 2=y.
                    """
                    for (v, cst, oidx) in vecs:
                        v2 = [2.0 * float(v[i]) for i in range(3)]
                        Tv = big("T", 3)
                        for (slot, ia, ib, ca, cb) in ((0, 1, 2, v2[1], v2[0]),
                                                       (1, 2, 3, v2[2], v2[1]),
                                                       (2, 3, 1, v2[0], v2[2])):
                            m = tile("m")
                            bal.ts_mult(m, WQv[:, ia, :], ca)
                            bal.stt(Tv[:, slot, :], WQv[:, ib, :], -cb, m, MULT, ADD)
                        A1v = big("A1", 3)
                        wb = WQv[:, 0:1, :].to_broadcast([P, 2, F])
                        bal.tt(A1v[:, 0:2, :], Tv[:, 1:3, :], wb, MULT, 2)
                        bal.tt(A1v[:, 2, :], Tv[:, 0, :], WQv[:, 0, :], MULT)
                        A2v = big("A2", 3)
                        bal.tt(A2v[:, 0:2, :], WQv[:, 2:4, :], Tv[:, 0:2, :], MULT, 2)
                        bal.tt(A2v[:, 2, :], WQv[:, 1, :], Tv[:, 2, :], MULT)
                        A4v = big("A4", 3)
                        bal.tt(A4v[:, 0, :], WQv[:, 3, :], Tv[:, 2, :], MULT)
                        bal.tt(A4v[:, 1:3, :], WQv[:, 1:3, :], Tv[:, 0:2, :], MULT, 2)
                        Rv = big("R", 3)
                        bal.tt(Rv, A1v, A2v, ADD, 3)
                        bal.tt(Rv, Rv, A4v, SUB, 3)
                        if oidx is None:
                            bal.tt(PaccV, PaccV, Rv, ADD, 3)
                        else:
                            for ci in range(3):
                                bal.stt(OSr[:, oidx + ci, :], Rv[:, ci, :],
                                        float(cst[ci]), PaccV[:, ci, :], ADD, ADD)

                # sites on body 1 (root)
                for sid in site_by_body.get(1, []):
                    emit_rot(WQv, [(sp[sid], sp[sid], 3 * sid)])

                # P init: += Rt(wq1) K_2
                emit_rot(WQv, [(K[0], None, None)])

                # ---- chain ----
                for j in range(2, m_max + 1):
                    h = j - 2
                    # lq = c*A + s*B into LQ block
                    LQv = big("lq", 4)
                    for i in range(4):
                        ti = tile(f"lqt{i}")
                        bal.ts_mult(ti, s3[:, h, :], float(Bq[h, i]))
                        bal.stt(LQv[:, i, :], c3[:, h, :], float(A[h, i]), ti, MULT, ADD)
                    # products: PA_a[b] = wq_a * lq_b  (4 broadcast block ops)
                    PAs = []
                    for a in range(4):
                        PAv = big(f"pa{a}", 4)
                        wqa = WQv[:, a:a + 1, :].to_broadcast([P, 4, F])
                        bal.tt(PAv, LQv, wqa, MULT, 4)
                        PAs.append(PAv)
                    # combines: nq = wq x lq via sign-grouped column pairs
                    NQv = big("wq", 4)
                    bal.tt(NQv[:, 1::2, :], PAs[0][:, 1::2, :], PAs[1][:, 0::2, :], ADD, 2)
                    bal.tt(NQv[:, 0::2, :], PAs[0][:, 0::2, :], PAs[1][:, 1::2, :], SUB, 2)
                    bal.tt(NQv[:, 1:3, :], NQv[:, 1:3, :], PAs[2][:, 3::-3, :], ADD, 2)
                    bal.tt(NQv[:, 0::3, :], NQv[:, 0::3, :], PAs[2][:, 2:0:-1, :], SUB, 2)
                    bal.tt(NQv[:, 2:4, :], NQv[:, 2:4, :], PAs[3][:, 1::-1, :], ADD, 2)
                    bal.tt(NQv[:, 0:2, :], NQv[:, 0:2, :], PAs[3][:, 3:1:-1, :], SUB, 2)
                    WQv = NQv
                    # sites on body j, then G update
                    vecs = []
                    for sid in site_by_body.get(j, []):
                        v = sp[sid] - jp[h]
                        vecs.append((v, Cpre[j] + v, 3 * sid))
                    if j < m_max:
                        vecs.append((G[h], None, None))
                    emit_rot(WQv, vecs)

                nc.sync.dma_start(out_d[:].rearrange("(p f) k -> p (f k)", p=P), OS[:])

    nc.compile()
    return nc


def _get_nc(inputs, loop_iters=None):
    key_src = b"".join(np.ascontiguousarray(np.asarray(inputs[k])).tobytes()
                       for k in ("body_pos", "body_quat", "hinge_axis", "jnt_pos",
                                 "site_pos", "body_parent", "site_body"))
    key = (hashlib.sha256(key_src).hexdigest(), loop_iters)
    if key not in _CACHE:
        _CACHE[key] = _build(inputs["body_pos"], inputs["body_quat"],
                             inputs["hinge_axis"], inputs["jnt_pos"],
                             inputs["site_pos"], inputs["body_parent"],
                             inputs["site_body"], loop_iters=loop_iters)
    return _CACHE[key]


def kernel(**inputs) -> np.ndarray:
    qpos = np.ascontiguousarray(np.asarray(inputs["qpos"], dtype=np.float32))
    assert qpos.shape == (B_FULL, NQ)
    nc = _get_nc(inputs)
    in_maps = [{"qpos": qpos[c * N:(c + 1) * N]} for c in range(NCORES)]
    res = run_bass_kernel_spmd(nc, in_maps, list(range(NCORES)))
    out = np.concatenate([res.results[c]["sites"] for c in range(NCORES)], axis=0)
    return out.reshape(B_FULL, NSITES, 3)


if __name__ == "__main__":
    import importlib.util
    spec = importlib.util.spec_from_file_location("reference", "/root/problem/reference.py")
    ref = importlib.util.module_from_spec(spec)
    spec.loader.exec_module(ref)
    inputs = {k: np.asarray(v) for k, v in ref.setup_inputs().items()}
    out = kernel(**inputs)
    print("out", out.shape, out.dtype)


# revision 17
# speedup vs baseline: 1.9521x; 1.4255x over previous
"""Differentiable-FK forward kernel for Trainium2 (8 NeuronCores, data-parallel).

Problem: batch B=131072 of kinematic chains (63 bodies: world, free root,
61 hinges), 16 sites gathered from bodies. Output [B, 16, 3] site positions.

Strategy: pure data-parallel across 8 cores (16384 rows each). Per core the
batch is laid out as [128 partitions x 128 free]. The hinge chain is evaluated
sequentially (61 quaternion composes). Positions use a telescoped accumulation

    out_s(m) = wp1 + Rt(wq_1)K_2 + sum_{j=2}^{m-1} Rt(wq_j) G_j
             + Rt(wq_m)(sp_s - jp_m) + CONST_s

where Rt(q)v = R(q)v - v is the pure-quadratic part of the rotation and all
constant vectors (G_j, CONST_s) are host-precomputed from the tiny tree
tensors and baked into instruction immediates. The kernel is recompiled (and
NEFF-cached) per unique set of tree constants; qpos is the only streamed
input.

Quaternion state lives in [128, 4*F] block tiles (component-major columns) so
the 16 per-body Hamilton products collapse to 4 broadcast tensor_tensor ops
and the 12 combine adds to 6 column-pair ops; rotation emits use a cyclically
ordered cross-product tile so each emit is ~15 instructions of [128, 2-3*F]
blocks. Work is spread across DVE / Pool / Act with a size-aware greedy
balancer using HW-calibrated per-op costs.
"""
import hashlib
import os
import numpy as np

# engine-mix tuning knob: scales Pool's modeled cost (higher -> less on Pool;
# 1e9 disables Pool entirely). HW A/B (fp16 core): Pool disabled = 443us/iter
# vs 565us with Pool -- GpSimd elementwise contends with DVE for SBUF ports
# and is slow besides, so it is net-negative here. Default: no Pool.
_POOL_FACTOR = float(os.environ.get("FK_POOL_FACTOR", "1e9"))

import concourse.bacc as bacc
import concourse.mybir as mybir
from concourse.tile import TileContext
from concourse.bass_utils import run_bass_kernel_spmd

F32 = mybir.dt.float32
F16 = mybir.dt.float16
# Core chain dtype: f16 halves DVE big-op time (390 vs 641 ns @[128,512]);
# boundaries (input, root normalize, output) stay f32. FK_F16=0 reverts.
DT = F16 if os.environ.get("FK_F16", "1") == "1" else F32
MULT = mybir.AluOpType.mult
ADD = mybir.AluOpType.add
SUB = mybir.AluOpType.subtract

B_FULL = 131072
NCORES = 8
N = B_FULL // NCORES          # 16384 rows per core
P = 128                       # partitions
F = N // P                    # 128 free
NBODY = 63
NH = NBODY - 2                # 61
NQ = 7 + NH                   # 68
NSITES = 16
OUTW = NSITES * 3             # 48

HALF_PI = float(np.pi / 2)

# HW-calibrated per-op costs (ns), indexed by free-size multiple k (k*128).
if DT is F16:
    DVE_TT = {1: 255.0, 2: 300.0, 3: 345.0, 4: 390.0}
    DVE_STT = 243.0
else:
    DVE_TT = {1: 229.0, 2: 366.0, 3: 504.0, 4: 641.0}
    DVE_STT = 288.0
POOL_TT = {1: 436.0, 2: 718.0, 3: 1000.0, 4: 1283.0}
DVE_TS = 197.0
ACT_TS = 326.0
DVE_CP = {1: 197.0, 2: 330.0, 3: 460.0, 4: 590.0}
ACT_CP = {1: 326.0, 2: 450.0, 3: 580.0, 4: 710.0}

_CACHE = {}


def _qmul_np(q1, q2):
    w1, x1, y1, z1 = [q1[..., i] for i in range(4)]
    w2, x2, y2, z2 = [q2[..., i] for i in range(4)]
    return np.stack([
        w1 * w2 - x1 * x2 - y1 * y2 - z1 * z2,
        w1 * x2 + x1 * w2 + y1 * z2 - z1 * y2,
        w1 * y2 - x1 * z2 + y1 * w2 + z1 * x2,
        w1 * z2 + x1 * y2 - y1 * x2 + z1 * w2,
    ], -1)


class Bal:
    """Greedy size-aware engine balancer across DVE / Pool / Act."""

    def __init__(self, nc):
        self.nc = nc
        self.load = {"dve": 0.0, "pool": 0.0, "act": 0.0}

    def tt(self, out, a, b, op, k=1):
        dve_c = DVE_TT[k]
        pool_c = POOL_TT[k] * _POOL_FACTOR
        if self.load["dve"] + dve_c <= self.load["pool"] + pool_c:
            self.load["dve"] += dve_c
            self.nc.vector.tensor_tensor(out, a, b, op)
        else:
            self.load["pool"] += pool_c
            self.nc.gpsimd.tensor_tensor(out, a, b, op)

    def stt(self, out, a, scal, b, op0, op1):
        self.load["dve"] += DVE_STT
        self.nc.vector.scalar_tensor_tensor(out, a, scal, b, op0, op1)

    def ts_mult(self, out, a, scal):
        if self.load["dve"] + DVE_TS <= self.load["act"] + ACT_TS:
            self.load["dve"] += DVE_TS
            self.nc.vector.tensor_scalar(out, a, float(scal), None, MULT)
        else:
            self.load["act"] += ACT_TS
            self.nc.scalar.activation(out, a, mybir.ActivationFunctionType.Copy,
                                      bias=0.0, scale=float(scal))

    def cp(self, out, a, k=1):
        if self.load["dve"] + DVE_CP[k] <= self.load["act"] + ACT_CP[k]:
            self.load["dve"] += DVE_CP[k]
            self.nc.vector.tensor_copy(out, a)
        else:
            self.load["act"] += ACT_CP[k]
            self.nc.scalar.activation(out, a, mybir.ActivationFunctionType.Copy,
                                      bias=0.0, scale=1.0)


def _build(body_pos, body_quat, hinge_axis, jnt_pos, site_pos,
           body_parent, site_body, loop_iters=None):
    # ---- host constant precompute ----
    parents = np.asarray(body_parent).astype(np.int64)
    sbody = np.asarray(site_body).astype(np.int64)
    assert np.array_equal(parents, np.maximum(np.arange(NBODY) - 1, 0)), \
        "kernel specialized for chain topology"

    A = np.asarray(body_quat[2:], np.float64)                      # [NH,4]
    Bq = _qmul_np(np.asarray(body_quat[2:], np.float64),
                  np.concatenate([np.zeros((NH, 1)), np.asarray(hinge_axis, np.float64)], -1))
    K = np.asarray(body_pos[2:], np.float64) + np.asarray(jnt_pos, np.float64)
    jp = np.asarray(jnt_pos, np.float64)
    sp = np.asarray(site_pos, np.float64)

    G = np.zeros((NH, 3))
    for h in range(NH):
        G[h] = (K[h + 1] if h + 1 < NH else 0.0) - jp[h]
    Cpre = np.zeros((NBODY, 3))
    acc = K[0].copy()
    for m in range(2, NBODY):
        Cpre[m] = acc
        acc = acc + G[m - 2]

    site_by_body = {}
    for s, m in enumerate(sbody):
        site_by_body.setdefault(int(m), []).append(s)
    m_max = max(site_by_body.keys())

    # ---- bass program ----
    nc = bacc.Bacc("TRN2")
    qpos_d = nc.dram_tensor("qpos", [N, NQ], F32, kind="ExternalInput")
    out_d = nc.dram_tensor("sites", [N, OUTW], F32, kind="ExternalOutput")

    Sin, Sqrt = (mybir.ActivationFunctionType.Sin,
                 mybir.ActivationFunctionType.Sqrt)

    with TileContext(nc) as tc:
        with tc.tile_pool(name="main", bufs=1) as pool, \
             tc.tile_pool(name="scratch", bufs=2) as sp_pool:

            bal = Bal(nc)

            # persistent allocations, shared across benchmark-loop iterations
            QP = pool.tile([P, F * NQ], F32)
            QPr = QP[:].rearrange("p (f k) -> p k f", k=NQ)       # [P, 68, F]
            s_all = pool.tile([P, NH * F], DT)
            c_all = pool.tile([P, NH * F], DT)
            s3 = s_all[:].rearrange("p (h f) -> p h f", h=NH)
            c3 = c_all[:].rearrange("p (h f) -> p h f", h=NH)
            halfpi = pool.tile([P, 1], F32, name="halfpi")
            nc.vector.memset(halfpi[:], HALF_PI)
            PaccB = pool.tile([P, 3 * F], DT, name="Pacc")
            PaccV = PaccB[:].rearrange("p (a f) -> p a f", a=3)
            OS = pool.tile([P, F * OUTW], F32)
            OSr = OS[:].rearrange("p (f k) -> p k f", k=OUTW)     # [P, 48, F]

            def big(tag, kf, dt=DT):
                t = sp_pool.tile([P, kf * F], dt, tag=tag, name=tag)
                return t[:].rearrange("p (a f) -> p a f", a=kf)

            def tile(tag, dt=DT):
                return sp_pool.tile([P, F], dt, tag=tag, name=tag)

            for _it in range(loop_iters or 1):
                nc.sync.dma_start(QP[:], qpos_d[:].rearrange("(p f) k -> p (f k)", p=P))
                ang = QPr[:, 7:NQ, :]                              # [P, 61, F]
                # s = sin(theta/2), c = sin(theta/2 + pi/2) = cos(theta/2)
                nc.scalar.activation(s3, ang, Sin, bias=0.0, scale=0.5)
                nc.scalar.activation(c3, ang, Sin, bias=halfpi[:], scale=0.5)
                bal.load["act"] += 2 * 6700.0  # bulk trig occupancy

                # ---- root: wq = normalize(qpos[:, 3:7]) ----
                rq = QPr[:, 3:7, :]                                # [P, 4, F] strided
                SQ = big("sq", 4, F32)
                bal.tt(SQ, rq, rq, MULT, 4)
                s01 = big("s01", 2, F32)
                bal.tt(s01, SQ[:, 0:2, :], SQ[:, 2:4, :], ADD, 2)
                n2 = tile("n2", F32)
                bal.tt(n2, s01[:, 0, :], s01[:, 1, :], ADD)
                rn = sp_pool.tile([P, F], F32, tag="rn", name="rn")
                nc.scalar.activation(rn, n2, Sqrt, bias=0.0, scale=1.0)
                nc.vector.reciprocal(rn, rn)
                WQv = big("wq", 4)
                rnb = rn[:].rearrange("p (a f) -> p a f", a=1).to_broadcast([P, 4, F])
                bal.tt(WQv, rq, rnb, MULT, 4)

                bal.cp(PaccV, QPr[:, 0:3, :], 3)

                def emit_rot(WQv, vecs):
                    """vecs = list of (v3, const3_or_None, out_idx).
                    out_idx None -> Pacc += Rt(q)v ; else OSr cols = Pacc + Rt(q)v + cst.
                    WQv components: w=0, x=1, y=2, z=3. T slots: 0=z, 1=x, 2=y.
                    """
                    for (v, cst, oidx) in vecs:
                        v2 = [2.0 * float(v[i]) for i in range(3)]
                        Tv = big("T", 3)
                        for (slot, ia, ib, ca, cb) in ((0, 1, 2, v2[1], v2[0]),
                                                       (1, 2, 3, v2[2], v2[1]),
                                                       (2, 3, 1, v2[0], v2[2])):
                            m = tile("m")
                            bal.ts_mult(m, WQv[:, ia, :], ca)
                            bal.stt(Tv[:, slot, :], WQv[:, ib, :], -cb, m, MULT, ADD)
                        A1v = big("A1", 3)
                        wb = WQv[:, 0:1, :].to_broadcast([P, 2, F])
                        bal.tt(A1v[:, 0:2, :], Tv[:, 1:3, :], wb, MULT, 2)
                        bal.tt(A1v[:, 2, :], Tv[:, 0, :], WQv[:, 0, :], MULT)
                        A2v = big("A2", 3)
                        bal.tt(A2v[:, 0:2, :], WQv[:, 2:4, :], Tv[:, 0:2, :], MULT, 2)
                        bal.tt(A2v[:, 2, :], WQv[:, 1, :], Tv[:, 2, :], MULT)
                        A4v = big("A4", 3)
                        bal.tt(A4v[:, 0, :], WQv[:, 3, :], Tv[:, 2, :], MULT)
                        bal.tt(A4v[:, 1:3, :], WQv[:, 1:3, :], Tv[:, 0:2, :], MULT, 2)
                        Rv = big("R", 3)
                        bal.tt(Rv, A1v, A2v, ADD, 3)
                        bal.tt(Rv, Rv, A4v, SUB, 3)
                        if oidx is None:
                            bal.tt(PaccV, PaccV, Rv, ADD, 3)
                        else:
                            for ci in range(3):
                                bal.stt(OSr[:, oidx + ci, :], Rv[:, ci, :],
                                        float(cst[ci]), PaccV[:, ci, :], ADD, ADD)

                # sites on body 1 (root)
                for sid in site_by_body.get(1, []):
                    emit_rot(WQv, [(sp[sid], sp[sid], 3 * sid)])

                # P init: += Rt(wq1) K_2
                emit_rot(WQv, [(K[0], None, None)])

                # ---- chain ----
                for j in range(2, m_max + 1):
                    h = j - 2
                    # lq = c*A + s*B into LQ block
                    LQv = big("lq", 4)
                    for i in range(4):
                        ti = tile(f"lqt{i}")
                        bal.ts_mult(ti, s3[:, h, :], float(Bq[h, i]))
                        bal.stt(LQv[:, i, :], c3[:, h, :], float(A[h, i]), ti, MULT, ADD)
                    # products: PA_a[b] = wq_a * lq_b  (4 broadcast block ops)
                    PAs = []
                    for a in range(4):
                        PAv = big(f"pa{a}", 4)
                        wqa = WQv[:, a:a + 1, :].to_broadcast([P, 4, F])
                        bal.tt(PAv, LQv, wqa, MULT, 4)
                        PAs.append(PAv)
                    # combines: nq = wq x lq via sign-grouped column pairs
                    NQv = big("wq", 4)
                    bal.tt(NQv[:, 1::2, :], PAs[0][:, 1::2, :], PAs[1][:, 0::2, :], ADD, 2)
                    bal.tt(NQv[:, 0::2, :], PAs[0][:, 0::2, :], PAs[1][:, 1::2, :], SUB, 2)
                    bal.tt(NQv[:, 1:3, :], NQv[:, 1:3, :], PAs[2][:, 3::-3, :], ADD, 2)
                    bal.tt(NQv[:, 0::3, :], NQv[:, 0::3, :], PAs[2][:, 2:0:-1, :], SUB, 2)
                    bal.tt(NQv[:, 2:4, :], NQv[:, 2:4, :], PAs[3][:, 1::-1, :], ADD, 2)
                    bal.tt(NQv[:, 0:2, :], NQv[:, 0:2, :], PAs[3][:, 3:1:-1, :], SUB, 2)
                    WQv = NQv
                    # sites on body j, then G update
                    vecs = []
                    for sid in site_by_body.get(j, []):
                        v = sp[sid] - jp[h]
                        vecs.append((v, Cpre[j] + v, 3 * sid))
                    if j < m_max:
                        vecs.append((G[h], None, None))
                    emit_rot(WQv, vecs)

                nc.sync.dma_start(out_d[:].rearrange("(p f) k -> p (f k)", p=P), OS[:])

    nc.compile()
    return nc


def _get_nc(inputs, loop_iters=None):
    key_src = b"".join(np.ascontiguousarray(np.asarray(inputs[k])).tobytes()
                       for k in ("body_pos", "body_quat", "hinge_axis", "jnt_pos",
                                 "site_pos", "body_parent", "site_body"))
    key = (hashlib.sha256(key_src).hexdigest(), loop_iters)
    if key not in _CACHE:
        _CACHE[key] = _build(inputs["body_pos"], inputs["body_quat"],
                             inputs["hinge_axis"], inputs["jnt_pos"],
                             inputs["site_pos"], inputs["body_parent"],
                             inputs["site_body"], loop_iters=loop_iters)
    return _CACHE[key]


def kernel(**inputs) -> np.ndarray:
    qpos = np.ascontiguousarray(np.asarray(inputs["qpos"], dtype=np.float32))
    assert qpos.shape == (B_FULL, NQ)
    nc = _get_nc(inputs)
    in_maps = [{"qpos": qpos[c * N:(c + 1) * N]} for c in range(NCORES)]
    res = run_bass_kernel_spmd(nc, in_maps, list(range(NCORES)))
    out = np.concatenate([res.results[c]["sites"] for c in range(NCORES)], axis=0)
    return out.reshape(B_FULL, NSITES, 3)


if __name__ == "__main__":
    import importlib.util
    spec = importlib.util.spec_from_file_location("reference", "/root/problem/reference.py")
    ref = importlib.util.module_from_spec(spec)
    spec.loader.exec_module(ref)
    inputs = {k: np.asarray(v) for k, v in ref.setup_inputs().items()}
    out = kernel(**inputs)
    print("out", out.shape, out.dtype)


# revision 18
# speedup vs baseline: 2.0178x; 1.0336x over previous
"""Differentiable-FK forward kernel for Trainium2 (8 NeuronCores, data-parallel).

Problem: batch B=131072 of kinematic chains (63 bodies: world, free root,
61 hinges), 16 sites gathered from bodies. Output [B, 16, 3] site positions.

Strategy: pure data-parallel across 8 cores (16384 rows each). Per core the
batch is laid out as [128 partitions x 128 free]. The hinge chain is evaluated
sequentially (61 quaternion composes). Positions use a telescoped accumulation

    out_s(m) = wp1 + Rt(wq_1)K_2 + sum_{j=2}^{m-1} Rt(wq_j) G_j
             + Rt(wq_m)(sp_s - jp_m) + CONST_s

where Rt(q)v = R(q)v - v is the pure-quadratic part of the rotation and all
constant vectors (G_j, CONST_s) are host-precomputed from the tiny tree
tensors and baked into instruction immediates. The kernel is recompiled (and
NEFF-cached) per unique set of tree constants; qpos is the only streamed
input.

Quaternion state lives in [128, 4*F] block tiles (component-major columns) so
the 16 per-body Hamilton products collapse to 4 broadcast tensor_tensor ops
and the 12 combine adds to 6 column-pair ops; rotation emits use a cyclically
ordered cross-product tile so each emit is ~15 instructions of [128, 2-3*F]
blocks. Work is spread across DVE / Pool / Act with a size-aware greedy
balancer using HW-calibrated per-op costs.
"""
import hashlib
import os
import numpy as np

# engine-mix tuning knob: scales Pool's modeled cost (higher -> less on Pool;
# 1e9 disables Pool entirely). HW A/B (fp16 core): Pool disabled = 443us/iter
# vs 565us with Pool -- GpSimd elementwise contends with DVE for SBUF ports
# and is slow besides, so it is net-negative here. Default: no Pool.
_POOL_FACTOR = float(os.environ.get("FK_POOL_FACTOR", "1e9"))

import concourse.bacc as bacc
import concourse.mybir as mybir
from concourse.tile import TileContext
from concourse.bass_utils import run_bass_kernel_spmd

F32 = mybir.dt.float32
F16 = mybir.dt.float16
# Core chain dtype: f16 halves DVE big-op time (390 vs 641 ns @[128,512]);
# boundaries (input, root normalize, output) stay f32. FK_F16=0 reverts.
DT = F16 if os.environ.get("FK_F16", "1") == "1" else F32
MULT = mybir.AluOpType.mult
ADD = mybir.AluOpType.add
SUB = mybir.AluOpType.subtract

B_FULL = 131072
NCORES = 8
N = B_FULL // NCORES          # 16384 rows per core
P = 128                       # partitions
F = N // P                    # 128 free
NBODY = 63
NH = NBODY - 2                # 61
NQ = 7 + NH                   # 68
NSITES = 16
OUTW = NSITES * 3             # 48

HALF_PI = float(np.pi / 2)

# HW-calibrated per-op costs (ns), indexed by free-size multiple k (k*128).
if DT is F16:
    DVE_TT = {1: 255.0, 2: 300.0, 3: 345.0, 4: 390.0}
    DVE_STT = 243.0
else:
    DVE_TT = {1: 229.0, 2: 366.0, 3: 504.0, 4: 641.0}
    DVE_STT = 288.0
POOL_TT = {1: 436.0, 2: 718.0, 3: 1000.0, 4: 1283.0}
DVE_TS = 197.0
ACT_TS = 326.0
DVE_CP = {1: 197.0, 2: 330.0, 3: 460.0, 4: 590.0}
ACT_CP = {1: 326.0, 2: 450.0, 3: 580.0, 4: 710.0}

_CACHE = {}


def _qmul_np(q1, q2):
    w1, x1, y1, z1 = [q1[..., i] for i in range(4)]
    w2, x2, y2, z2 = [q2[..., i] for i in range(4)]
    return np.stack([
        w1 * w2 - x1 * x2 - y1 * y2 - z1 * z2,
        w1 * x2 + x1 * w2 + y1 * z2 - z1 * y2,
        w1 * y2 - x1 * z2 + y1 * w2 + z1 * x2,
        w1 * z2 + x1 * y2 - y1 * x2 + z1 * w2,
    ], -1)


class Bal:
    """Greedy size-aware engine balancer across DVE / Pool / Act."""

    def __init__(self, nc):
        self.nc = nc
        self.load = {"dve": 0.0, "pool": 0.0, "act": 0.0}

    def tt(self, out, a, b, op, k=1):
        dve_c = DVE_TT[k]
        pool_c = POOL_TT[k] * _POOL_FACTOR
        if self.load["dve"] + dve_c <= self.load["pool"] + pool_c:
            self.load["dve"] += dve_c
            self.nc.vector.tensor_tensor(out, a, b, op)
        else:
            self.load["pool"] += pool_c
            self.nc.gpsimd.tensor_tensor(out, a, b, op)

    def stt(self, out, a, scal, b, op0, op1):
        self.load["dve"] += DVE_STT
        self.nc.vector.scalar_tensor_tensor(out, a, scal, b, op0, op1)

    def ts_mult(self, out, a, scal):
        if self.load["dve"] + DVE_TS <= self.load["act"] + ACT_TS:
            self.load["dve"] += DVE_TS
            self.nc.vector.tensor_scalar(out, a, float(scal), None, MULT)
        else:
            self.load["act"] += ACT_TS
            self.nc.scalar.activation(out, a, mybir.ActivationFunctionType.Copy,
                                      bias=0.0, scale=float(scal))

    def cp(self, out, a, k=1):
        if self.load["dve"] + DVE_CP[k] <= self.load["act"] + ACT_CP[k]:
            self.load["dve"] += DVE_CP[k]
            self.nc.vector.tensor_copy(out, a)
        else:
            self.load["act"] += ACT_CP[k]
            self.nc.scalar.activation(out, a, mybir.ActivationFunctionType.Copy,
                                      bias=0.0, scale=1.0)


def _build(body_pos, body_quat, hinge_axis, jnt_pos, site_pos,
           body_parent, site_body, loop_iters=None):
    # ---- host constant precompute ----
    parents = np.asarray(body_parent).astype(np.int64)
    sbody = np.asarray(site_body).astype(np.int64)
    assert np.array_equal(parents, np.maximum(np.arange(NBODY) - 1, 0)), \
        "kernel specialized for chain topology"

    A = np.asarray(body_quat[2:], np.float64)                      # [NH,4]
    Bq = _qmul_np(np.asarray(body_quat[2:], np.float64),
                  np.concatenate([np.zeros((NH, 1)), np.asarray(hinge_axis, np.float64)], -1))
    K = np.asarray(body_pos[2:], np.float64) + np.asarray(jnt_pos, np.float64)
    jp = np.asarray(jnt_pos, np.float64)
    sp = np.asarray(site_pos, np.float64)

    G = np.zeros((NH, 3))
    for h in range(NH):
        G[h] = (K[h + 1] if h + 1 < NH else 0.0) - jp[h]
    Cpre = np.zeros((NBODY, 3))
    acc = K[0].copy()
    for m in range(2, NBODY):
        Cpre[m] = acc
        acc = acc + G[m - 2]

    site_by_body = {}
    for s, m in enumerate(sbody):
        site_by_body.setdefault(int(m), []).append(s)
    m_max = max(site_by_body.keys())

    # ---- bass program ----
    nc = bacc.Bacc("TRN2")
    qpos_d = nc.dram_tensor("qpos", [N, NQ], F32, kind="ExternalInput")
    out_d = nc.dram_tensor("sites", [N, OUTW], F32, kind="ExternalOutput")

    Sin, Sqrt = (mybir.ActivationFunctionType.Sin,
                 mybir.ActivationFunctionType.Sqrt)

    with TileContext(nc) as tc:
        with tc.tile_pool(name="main", bufs=1) as pool, \
             tc.tile_pool(name="scratch", bufs=int(os.environ.get("FK_BUFS", "3"))) as sp_pool:

            bal = Bal(nc)

            # persistent allocations, shared across benchmark-loop iterations
            QP = pool.tile([P, F * NQ], F32)
            QPr = QP[:].rearrange("p (f k) -> p k f", k=NQ)       # [P, 68, F]
            s_all = pool.tile([P, NH * F], DT)
            c_all = pool.tile([P, NH * F], DT)
            s3 = s_all[:].rearrange("p (h f) -> p h f", h=NH)
            c3 = c_all[:].rearrange("p (h f) -> p h f", h=NH)
            halfpi = pool.tile([P, 1], F32, name="halfpi")
            nc.vector.memset(halfpi[:], HALF_PI)
            PaccB = pool.tile([P, 3 * F], DT, name="Pacc")
            PaccV = PaccB[:].rearrange("p (a f) -> p a f", a=3)
            OS = pool.tile([P, F * OUTW], F32)
            OSr = OS[:].rearrange("p (f k) -> p k f", k=OUTW)     # [P, 48, F]

            def big(tag, kf, dt=DT):
                t = sp_pool.tile([P, kf * F], dt, tag=tag, name=tag)
                return t[:].rearrange("p (a f) -> p a f", a=kf)

            def tile(tag, dt=DT):
                return sp_pool.tile([P, F], dt, tag=tag, name=tag)

            for _it in range(loop_iters or 1):
                nc.sync.dma_start(QP[:], qpos_d[:].rearrange("(p f) k -> p (f k)", p=P))
                ang = QPr[:, 7:NQ, :]                              # [P, 61, F]
                # s = sin(theta/2), c = sin(theta/2 + pi/2) = cos(theta/2)
                nc.scalar.activation(s3, ang, Sin, bias=0.0, scale=0.5)
                nc.scalar.activation(c3, ang, Sin, bias=halfpi[:], scale=0.5)
                bal.load["act"] += 2 * 6700.0  # bulk trig occupancy

                # ---- root: wq = normalize(qpos[:, 3:7]) ----
                rq = QPr[:, 3:7, :]                                # [P, 4, F] strided
                SQ = big("sq", 4, F32)
                bal.tt(SQ, rq, rq, MULT, 4)
                s01 = big("s01", 2, F32)
                bal.tt(s01, SQ[:, 0:2, :], SQ[:, 2:4, :], ADD, 2)
                n2 = tile("n2", F32)
                bal.tt(n2, s01[:, 0, :], s01[:, 1, :], ADD)
                rn = sp_pool.tile([P, F], F32, tag="rn", name="rn")
                nc.scalar.activation(rn, n2, Sqrt, bias=0.0, scale=1.0)
                nc.vector.reciprocal(rn, rn)
                WQv = big("wq", 4)
                rnb = rn[:].rearrange("p (a f) -> p a f", a=1).to_broadcast([P, 4, F])
                bal.tt(WQv, rq, rnb, MULT, 4)

                bal.cp(PaccV, QPr[:, 0:3, :], 3)

                def emit_rot(WQv, vecs):
                    """vecs = list of (v3, const3_or_None, out_idx).
                    out_idx None -> Pacc += Rt(q)v ; else OSr cols = Pacc + Rt(q)v + cst.
                    WQv components: w=0, x=1, y=2, z=3. T slots: 0=z, 1=x, 2=y.
                    """
                    for (v, cst, oidx) in vecs:
                        v2 = [2.0 * float(v[i]) for i in range(3)]
                        Tv = big("T", 3)
                        for (slot, ia, ib, ca, cb) in ((0, 1, 2, v2[1], v2[0]),
                                                       (1, 2, 3, v2[2], v2[1]),
                                                       (2, 3, 1, v2[0], v2[2])):
                            m = tile("m")
                            bal.ts_mult(m, WQv[:, ia, :], ca)
                            bal.stt(Tv[:, slot, :], WQv[:, ib, :], -cb, m, MULT, ADD)
                        A1v = big("A1", 3)
                        wb = WQv[:, 0:1, :].to_broadcast([P, 2, F])
                        bal.tt(A1v[:, 0:2, :], Tv[:, 1:3, :], wb, MULT, 2)
                        bal.tt(A1v[:, 2, :], Tv[:, 0, :], WQv[:, 0, :], MULT)
                        A2v = big("A2", 3)
                        bal.tt(A2v[:, 0:2, :], WQv[:, 2:4, :], Tv[:, 0:2, :], MULT, 2)
                        bal.tt(A2v[:, 2, :], WQv[:, 1, :], Tv[:, 2, :], MULT)
                        A4v = big("A4", 3)
                        bal.tt(A4v[:, 0, :], WQv[:, 3, :], Tv[:, 2, :], MULT)
                        bal.tt(A4v[:, 1:3, :], WQv[:, 1:3, :], Tv[:, 0:2, :], MULT, 2)
                        Rv = big("R", 3)
                        bal.tt(Rv, A1v, A2v, ADD, 3)
                        bal.tt(Rv, Rv, A4v, SUB, 3)
                        if oidx is None:
                            bal.tt(PaccV, PaccV, Rv, ADD, 3)
                        else:
                            for ci in range(3):
                                bal.stt(OSr[:, oidx + ci, :], Rv[:, ci, :],
                                        float(cst[ci]), PaccV[:, ci, :], ADD, ADD)

                # sites on body 1 (root)
                for sid in site_by_body.get(1, []):
                    emit_rot(WQv, [(sp[sid], sp[sid], 3 * sid)])

                # P init: += Rt(wq1) K_2
                emit_rot(WQv, [(K[0], None, None)])

                # ---- chain ----
                for j in range(2, m_max + 1):
                    h = j - 2
                    # lq = c*A + s*B into LQ block
                    LQv = big("lq", 4)
                    for i in range(4):
                        ti = tile(f"lqt{i}")
                        bal.ts_mult(ti, s3[:, h, :], float(Bq[h, i]))
                        bal.stt(LQv[:, i, :], c3[:, h, :], float(A[h, i]), ti, MULT, ADD)
                    # products: PA_a[b] = wq_a * lq_b  (4 broadcast block ops)
                    PAs = []
                    for a in range(4):
                        PAv = big(f"pa{a}", 4)
                        wqa = WQv[:, a:a + 1, :].to_broadcast([P, 4, F])
                        bal.tt(PAv, LQv, wqa, MULT, 4)
                        PAs.append(PAv)
                    # combines: nq = wq x lq via sign-grouped column pairs
                    NQv = big("wq", 4)
                    bal.tt(NQv[:, 1::2, :], PAs[0][:, 1::2, :], PAs[1][:, 0::2, :], ADD, 2)
                    bal.tt(NQv[:, 0::2, :], PAs[0][:, 0::2, :], PAs[1][:, 1::2, :], SUB, 2)
                    bal.tt(NQv[:, 1:3, :], NQv[:, 1:3, :], PAs[2][:, 3::-3, :], ADD, 2)
                    bal.tt(NQv[:, 0::3, :], NQv[:, 0::3, :], PAs[2][:, 2:0:-1, :], SUB, 2)
                    bal.tt(NQv[:, 2:4, :], NQv[:, 2:4, :], PAs[3][:, 1::-1, :], ADD, 2)
                    bal.tt(NQv[:, 0:2, :], NQv[:, 0:2, :], PAs[3][:, 3:1:-1, :], SUB, 2)
                    WQv = NQv
                    # sites on body j, then G update
                    vecs = []
                    for sid in site_by_body.get(j, []):
                        v = sp[sid] - jp[h]
                        vecs.append((v, Cpre[j] + v, 3 * sid))
                    if j < m_max:
                        vecs.append((G[h], None, None))
                    emit_rot(WQv, vecs)

                nc.sync.dma_start(out_d[:].rearrange("(p f) k -> p (f k)", p=P), OS[:])

    nc.compile()
    return nc


def _get_nc(inputs, loop_iters=None):
    key_src = b"".join(np.ascontiguousarray(np.asarray(inputs[k])).tobytes()
                       for k in ("body_pos", "body_quat", "hinge_axis", "jnt_pos",
                                 "site_pos", "body_parent", "site_body"))
    key = (hashlib.sha256(key_src).hexdigest(), loop_iters)
    if key not in _CACHE:
        _CACHE[key] = _build(inputs["body_pos"], inputs["body_quat"],
                             inputs["hinge_axis"], inputs["jnt_pos"],
                             inputs["site_pos"], inputs["body_parent"],
                             inputs["site_body"], loop_iters=loop_iters)
    return _CACHE[key]


def kernel(**inputs) -> np.ndarray:
    qpos = np.ascontiguousarray(np.asarray(inputs["qpos"], dtype=np.float32))
    assert qpos.shape == (B_FULL, NQ)
    nc = _get_nc(inputs)
    in_maps = [{"qpos": qpos[c * N:(c + 1) * N]} for c in range(NCORES)]
    res = run_bass_kernel_spmd(nc, in_maps, list(range(NCORES)))
    out = np.concatenate([res.results[c]["sites"] for c in range(NCORES)], axis=0)
    return out.reshape(B_FULL, NSITES, 3)


if __name__ == "__main__":
    import importlib.util
    spec = importlib.util.spec_from_file_location("reference", "/root/problem/reference.py")
    ref = importlib.util.module_from_spec(spec)
    spec.loader.exec_module(ref)
    inputs = {k: np.asarray(v) for k, v in ref.setup_inputs().items()}
    out = kernel(**inputs)
    print("out", out.shape, out.dtype)
